# revision 54
# baseline (speedup 1.0000x reference)
"""Trainium2 Bass kernel for nn_MultiHeadAttention (B=2, S=2048, E=1024, H=16, D=64).

Sharding: 8 cores = 2 batches x 4 head-groups (4 heads / core, d_local=256).
Each core computes, for its (batch b, head group g):
    q = Xq[b] @ Wq[:, hs]*0.125 + bq[hs]*0.125        (transposed layout QT [256, S])
    k = Xk[b] @ Wk[:, hs] + bk[hs]                    (transposed layout KT [256, S])
    v = Xv[b] @ Wv[:, hs] + bv[hs]                    (natural layout, 65-strided + ones col)
    per head: scores^T = K_h @ Q_h^T  -> exp (ACT) -> Z|denom = expW^T.T @ [V_h|1]
    Z normalized per-partition, PE-transposed to ZT [256, S]
    partial_out = Z @ Wo[hs, :]                       ([S, E] fp32, host sums over g)
Host: transposes/casts inputs to bf16, sums the 4 partials per batch, adds bo.

Schedule: the exp stream (128 score-tile exps on ACT, ~1.15us each) is the
critical resource; emission is an exp-paced "backbone" of 8 groups of 16 kt
slots, (pair, qc) interleaved (0,0),(1,0),(0,1),(1,1),... with all other PE
work (QKV projections both pairs per x load, AV, transposes, out-proj) pulled
between slots from a FIFO of generators plus explicit dependency guards.

Self-contained: hardcodes all shapes; requires only concourse (+ml_dtypes/numpy).
"""

import sys
import types

import numpy as np
import ml_dtypes

import concourse.bass as bass  # noqa: F401  (bass types used via tile/bacc)
import concourse.mybir as mybir
import concourse.tile as tile
from concourse import bacc
from concourse import bass_utils
from concourse.masks import make_identity

BF16 = mybir.dt.bfloat16
F32 = mybir.dt.float32
AF = mybir.ActivationFunctionType

B, S, E = 2, 2048, 1024
H, D = 16, 64
N_CORES = 8
HL = 4          # heads per core
DL = HL * D     # 256 local d
NPAIR = 2       # head pairs per core
KT_TILES = S // 128   # 16
QC = 4          # q chunks of 512
ET = E // 128   # 8 e-tiles


def _install_ntff_hook():
    """Register the axon NTFF profiling hook if the image's antenv lacks it."""
    try:
        import antenv  # noqa
        if 'antenv.axon_hooks' in sys.modules:
            return
        mod = types.ModuleType('antenv.axon_hooks')
        _hook = [None]
        mod.set_axon_ntff_profile_hook = lambda h: _hook.__setitem__(0, h)
        mod.get_axon_ntff_profile_hook = lambda: _hook[0]
        sys.modules['antenv.axon_hooks'] = mod
        setattr(antenv, 'axon_hooks', mod)
        try:
            from trn_agent_boot.trn_boot import _ntff_profile_via_ctypes
            h = _ntff_profile_via_ctypes('/opt/axon/libaxon_pjrt.so')
            if h is not None:
                mod.set_axon_ntff_profile_hook(h)
        except Exception:
            pass
    except Exception:
        pass


def build_kernel():
    nc = bacc.Bacc("TRN2", target_bir_lowering=False, debug=False,
                   enable_asserts=True, num_devices=N_CORES)

    # all inputs pre-arranged on host to be contiguous for their SBUF tiles;
    # x layout [sc, half, partition, c, j] makes each half-tile DMA fully
    # contiguous (4KB per partition row) for efficient HBM bursts
    xq_ap = nc.dram_tensor("xq_t", [QC, 2, 128, ET // 2, 512], BF16, kind="ExternalInput").ap()
    xk_ap = nc.dram_tensor("xk_t", [QC, 2, 128, ET // 2, 512], BF16, kind="ExternalInput").ap()
    xv_ap = nc.dram_tensor("xv_t", [QC, 2, 128, ET // 2, 512], BF16, kind="ExternalInput").ap()
    wq_ap = nc.dram_tensor("wq", [128, 2, ET, 128], BF16, kind="ExternalInput").ap()
    wk_ap = nc.dram_tensor("wk", [128, 2, ET, 128], BF16, kind="ExternalInput").ap()
    wv_ap = nc.dram_tensor("wv", [128, ET, HL * 65], BF16, kind="ExternalInput").ap()
    bq_ap = nc.dram_tensor("bq", [128, 2], F32, kind="ExternalInput").ap()
    bk_ap = nc.dram_tensor("bk", [128, 2], F32, kind="ExternalInput").ap()
    bvf_ap = nc.dram_tensor("bvf", [128, HL * 65], BF16, kind="ExternalInput").ap()
    wo_ap = nc.dram_tensor("wo", [128, 2, E], BF16, kind="ExternalInput").ap()
    out_ap = nc.dram_tensor("out_p", [S, E], BF16, kind="ExternalOutput").ap()

    from contextlib import ExitStack
    with tile.TileContext(nc) as tc, ExitStack() as ctx:
        wpool = ctx.enter_context(tc.tile_pool(name="w", bufs=1))
        xtp = ctx.enter_context(tc.tile_pool(name="xt", bufs=12))
        big = ctx.enter_context(tc.tile_pool(name="big", bufs=1))
        expp = ctx.enter_context(tc.tile_pool(name="expp", bufs=3))
        znp = ctx.enter_context(tc.tile_pool(name="znp", bufs=2))
        smal = ctx.enter_context(tc.tile_pool(name="small", bufs=2))
        stg = ctx.enter_context(tc.tile_pool(name="stg", bufs=4))
        pscore = ctx.enter_context(tc.tile_pool(name="pscore", bufs=2, space="PSUM"))
        pav = ctx.enter_context(tc.tile_pool(name="pav", bufs=1, space="PSUM"))
        ptr = ctx.enter_context(tc.tile_pool(name="ptr", bufs=1, space="PSUM"))
        ppo = ctx.enter_context(tc.tile_pool(name="ppo", bufs=2, space="PSUM"))

        # ---- persistent weights / constants ----
        wq_sb = wpool.tile([128, 2, ET, 128], BF16, tag="wq")
        wk_sb = wpool.tile([128, 2, ET, 128], BF16, tag="wk")
        wv_sb = wpool.tile([128, ET, HL * 65], BF16, tag="wv")
        wo_sb = wpool.tile([128, 2, E], BF16, tag="wo")
        bq_sb = wpool.tile([128, 2], F32, tag="bq")
        bk_sb = wpool.tile([128, 2], F32, tag="bk")
        bvf_sb = wpool.tile([128, HL * 65], BF16, tag="bvf")
        ident = wpool.tile([128, 128], BF16, tag="ident")

        make_identity(nc, ident[:])

        QT = big.tile([128, NPAIR, S], BF16, tag="QT")
        KT = big.tile([128, NPAIR, S], BF16, tag="KT")
        Vones = big.tile([128, KT_TILES, HL, 65], BF16, tag="Vones")
        ZT = big.tile([128, NPAIR, S], BF16, tag="ZT")

        def load_xt_half(ap, sc, hf, chunks=1):
            # chunks>1: split across DMA queues for more head bandwidth
            t = xtp.tile([128, ET // 2, 512], BF16, tag="xt", name="xt")
            step = 4 // chunks
            for c in range(chunks):
                nc.sync.dma_start(
                    t[:, c * step:(c + 1) * step, :],
                    ap[sc][hf][:, c * step:(c + 1) * step, :])
            return t

        def load_xt_sc(ap, sc, chunks=1):
            return (load_xt_half(ap, sc, 0, chunks),
                    load_xt_half(ap, sc, 1, chunks))

        def xe(x_pair, e):
            return x_pair[e // 4][:, e % 4, :]

        # prefetch caches: sc -> (half0, half1)
        xq_c, xk_c, xv_c = {}, {}, {}

        def prefetch(cache, ap, sc, chunks=1):
            if sc not in cache:
                cache[sc] = load_xt_sc(ap, sc, chunks)

        def take(cache, ap, sc):
            prefetch(cache, ap, sc)
            return cache.pop(sc)

        # Q/K projections as ONE sequential generator (so at most one live
        # ppo tile is held across yields), ordered to match x DMA arrival:
        # q0 first (unlocks the backbone), then all of xk, then q1-q3.
        # ONE merged projection generator, fine-grained yields (2 MMs per
        # yield) for smooth exp pacing. Order: q0p0/k0p0 first (unlocks the
        # backbone), then the rest of k, then q1-3. Strict-FIFO draining plus
        # guards that only ever touch THIS generator keep ppo single-user
        # until it is exhausted (gv runs only after, gop only after gv).
        # part 'a' ([q0/k0 interleaved per pair, then k1-3]) feeds the early
        # backbone; part 'b' ([q1-3]) sits AFTER gv in the FIFO so the V
        # projection (needed by the first AV) is emitted before q1-3.
        QK_SEQ_A = [('q', 0, 0), ('k', 0, 0), ('q', 0, 1), ('k', 0, 1),
                    ('k', 1, 0), ('k', 1, 1), ('k', 2, 0), ('k', 2, 1),
                    ('k', 3, 0), ('k', 3, 1)]
        QK_SEQ_B = [('q', 1, 0), ('q', 1, 1), ('q', 2, 0), ('q', 2, 1),
                    ('q', 3, 0), ('q', 3, 1)]
        prog = {'a': 0, 'b': 0}   # completed (tensor, sc, pair) units

        def need_qt(p, qc):
            if qc == 0:
                return 'a', 1 + 2 * p          # q0p0=1, q0p1=3
            return 'b', 2 * (qc - 1) + p + 1

        def need_kt(p, sc):
            if sc == 0:
                return 'a', 2 + 2 * p          # k0p0=2, k0p1=4
            return 'a', 4 + 2 * (sc - 1) + p + 1

        def gen_projqk(key, seq):
            units = 0
            for which, sc, p in seq:
                if which == 'q':
                    cache, x_ap, dst, w_sb, b_sb = xq_c, xq_ap, QT, wq_sb, bq_sb
                else:
                    cache, x_ap, dst, w_sb, b_sb = xk_c, xk_ap, KT, wk_sb, bk_sb
                prefetch(cache, x_ap, sc)
                x_sc = cache[sc]
                ps = ppo.tile([128, 512], F32, tag="ppo", name="ps")
                for e in range(ET):
                    nc.tensor.matmul(
                        ps[:], w_sb[:, p, e, :], xe(x_sc, e),
                        start=(e == 0), stop=(e == ET - 1))
                    if e % 2 == 1:
                        yield
                nc.vector.tensor_scalar_add(
                    dst[:, p, sc * 512:(sc + 1) * 512], ps[:], b_sb[:, p:p + 1])
                units += 1
                prog[key] = units
                yield

        v_prog = [0]   # completed V st-units (16 = Vones fully written)

        def gen_projv():
            # single-pass V projection (all 4 heads, N=260); bias added on
            # DVE. ATOMIC st-units (no yield while the ppo tile is live).
            for vsc in range(QC):
                x_sc = take(xv_c, xv_ap, vsc)
                if vsc + 1 < QC:
                    prefetch(xv_c, xv_ap, vsc + 1)
                yield
                for sti in range(4):
                    st = vsc * 4 + sti
                    ps = ppo.tile([128, HL * 65], F32, tag="ppo", name="ps")
                    for e in range(ET):
                        nc.tensor.matmul(
                            ps[:], xe(x_sc, e)[:, sti * 128:(sti + 1) * 128],
                            wv_sb[:, e, :],
                            start=(e == 0), stop=(e == ET - 1))
                    nc.vector.tensor_add(
                        Vones[:, st],
                        ps[:].rearrange("p (h d) -> p h d", h=HL),
                        bvf_sb[:].rearrange("p (h d) -> p h d", h=HL))
                    v_prog[0] += 1
                    yield

        def gen_av(p, qc, et):
            # AV + normalize + transpose for one (pair, q-chunk)
            zn = znp.tile([128, 4, 2, D], BF16, tag="zn", name="zn")
            for h in range(2):
                avp = pav.tile([128, 4, 65], F32, tag="av", name="avp")
                # qt-outer: interleaved accumulation groups in one PSUM bank
                # are NOT allowed (each group's start clears the whole bank's
                # has_written bits) — a qt group must fully precede the next.
                for qt in range(4):
                    for kt in range(KT_TILES):
                        nc.tensor.matmul(
                            avp[:, qt, :],
                            et[:, kt, h, qt * 128:(qt + 1) * 128],
                            Vones[:, kt, 2 * p + h, :],
                            start=(kt == 0), stop=(kt == KT_TILES - 1))
                        if kt % 8 == 7:
                            yield
                rc = smal.tile([128, 4, 1], F32, tag="rc", name="rc")
                nc.vector.reciprocal(rc[:], avp[:, :, 64:65])
                nc.vector.tensor_mul(zn[:, :, h, :], avp[:, :, 0:D],
                                     rc[:].to_broadcast([128, 4, D]))
                yield
            for qt in range(4):
                tp = ptr.tile([128, 128], BF16, tag="tr", name="tp")
                nc.tensor.transpose(tp[:], zn[:, qt], ident[:])
                nc.vector.tensor_copy(
                    ZT[:, p, qc * 512 + qt * 128: qc * 512 + (qt + 1) * 128], tp[:])
                if qt % 2 == 1:
                    yield

        def gen_outproj(sts, act_evict=False):
            for st in sts:
                stt = stg.tile([128, 2, 512], BF16, tag="stg", name="stt")
                for ec in range(2):
                    ps = ppo.tile([128, 512], F32, tag="ppo", name="ps")
                    for dt_ in range(2):
                        nc.tensor.matmul(
                            ps[:], ZT[:, dt_, st * 128:(st + 1) * 128],
                            wo_sb[:, dt_, ec * 512:(ec + 1) * 512],
                            start=(dt_ == 0), stop=(dt_ == 1))
                    if act_evict and ec == 1:
                        nc.scalar.copy(stt[:, ec], ps[:])
                    else:
                        nc.vector.tensor_copy(stt[:, ec], ps[:])
                    rows = out_ap[st * 128:(st + 1) * 128]
                    # last two st-units: ec1 DMA rides the idle scalar ring so
                    # the final two output chunks drain in parallel
                    ring = nc.scalar if (act_evict and st >= 14 and ec == 1) \
                        else nc.sync
                    ring.dma_start(rows[:, ec * 512:(ec + 1) * 512],
                                   stt[:, ec])
                    yield

        def scores_kts(p, qc, et, kts):
            for kt in kts:
                sc_t = pscore.tile([128, 2, 512], F32, tag="sc")
                for h in range(2):
                    nc.tensor.matmul(
                        sc_t[:, h, :],
                        KT[64 * h:64 * (h + 1), p, kt * 128:(kt + 1) * 128],
                        QT[64 * h:64 * (h + 1), p, qc * 512:(qc + 1) * 512],
                        start=True, stop=True, tile_position=(64 * h, 0))
                nc.scalar.activation(et[:, kt], sc_t[:], AF.Exp)

        def new_et():
            return expp.tile([128, KT_TILES, 2, 512], BF16, tag="expT", name="et")

        def drain(g, n=10 ** 9):
            """Pull generator g up to n times; True if exhausted."""
            for _ in range(n):
                if next(g, StopIteration) is StopIteration:
                    return True
            return False

        # filler FIFO: entries (generator, est_us_per_yield); strict head-first
        # draining so generators holding PSUM tiles never interleave. ALL
        # emission (pump + guards) is charged against one credit ledger so
        # per-slot PE work stays uniform and nothing pools into the tail.
        fifo = []
        credit = [0.0]
        GQK_COST = 0.45

        def pump(budget_us):
            credit[0] = min(credit[0] + budget_us, 3.2)
            while credit[0] > 0 and fifo:
                gen, cost = fifo[0]
                if drain(gen, 1):
                    fifo.pop(0)
                else:
                    credit[0] -= cost

        def fifo_drop(gen):
            for i, (g, _) in enumerate(fifo):
                if g is gen:
                    fifo.pop(i)
                    return

        def ensure(need):
            key, units = need
            gen = gqk_a if key == 'a' else gqk_b
            while prog[key] < units:
                if drain(gen, 1):
                    break
                credit[0] -= GQK_COST

        # ---- emission ----
        # ACT table preload: dummy exp at t=0 so the ~2.7us exp table load
        # hides under the DMA lead-in (first real exp fires ~6us in).
        dummy = wpool.tile([1, 2], BF16, tag="dummy")
        nc.scalar.activation(dummy[:], ident[0:1, 0:2], AF.Exp)

        # weights on the Activation HWDGE ring; biases FIRST (tiny, and the
        # first proj evicts block on them), then pair-0 weights.
        nc.scalar.dma_start(bq_sb[:], bq_ap[:])
        nc.scalar.dma_start(bk_sb[:], bk_ap[:])
        nc.scalar.dma_start(wq_sb[:, 0], wq_ap[:, 0])
        nc.scalar.dma_start(wk_sb[:, 0], wk_ap[:, 0])

        # Head x DMA: the early window is descriptor-rate-bound per DGE
        # ring, so spread xq0/xk0 across THREE rings (sync; scalar after the
        # small pair-0 weights; gpsimd) to parallelize descriptor
        # generation. These tiles are fresh (no WAR), so a waiting DMA can
        # never block queued engine work on those rings.
        def load_head_half_sc(ap, sc, hf, eng):
            t = xtp.tile([128, ET // 2, 512], BF16, tag="xt", name="xt")
            for c in range(4):
                eng.dma_start(t[:, c:c + 1, :], ap[sc][hf][:, c:c + 1, :])
            return t

        def load_head_half(ap, hf, eng):
            return load_head_half_sc(ap, 0, hf, eng)

        xq_c[0] = (load_head_half(xq_ap, 0, nc.sync),
                   load_head_half(xq_ap, 1, nc.sync))
        xk_c[0] = (load_head_half(xk_ap, 0, nc.scalar),
                   load_head_half(xk_ap, 1, nc.gpsimd))

        # rest of the weights after the head-critical xk0h0 on the scalar ring
        nc.scalar.dma_start(wq_sb[:, 1], wq_ap[:, 1])
        nc.scalar.dma_start(wk_sb[:, 1], wk_ap[:, 1])
        nc.scalar.dma_start(wv_sb[:], wv_ap[:])
        nc.scalar.dma_start(bvf_sb[:], bvf_ap[:])
        nc.scalar.dma_start(wo_sb[:], wo_ap[:])

        # xk1 rides the otherwise-idle gpsimd ring so kt8 of group 0 isn't
        # waiting behind xq0 on the sync ring
        xk_c[1] = (load_head_half_sc(xk_ap, 1, 0, nc.gpsimd),
                   load_head_half_sc(xk_ap, 1, 1, nc.gpsimd))
        prefetch(xk_c, xk_ap, 2)
        prefetch(xk_c, xk_ap, 3)

        # PE warmup (HAM ramp): matmuls on a zeroed tile spanning the DMA
        # lead-in so the first projections run at full clock.
        warm = wpool.tile([128, 256], BF16, tag="warm")
        nc.vector.memset(warm[:], 0.0)
        wps = ppo.tile([128, 512], F32, tag="ppo")
        for i in range(24):
            nc.tensor.matmul(wps[:, 0:256], warm[:, 0:128], warm[:],
                             start=(i == 0), stop=(i == 23))

        gqk_a = gen_projqk('a', QK_SEQ_A)
        gqk_b = gen_projqk('b', QK_SEQ_B)
        gv = gen_projv()

        ensure(need_qt(0, 0))             # QT pair-0 qc0
        credit[0] = 0.0
        fifo.append((gqk_a, GQK_COST))
        fifo.append((gv, 1.25))
        fifo.append((gqk_b, GQK_COST))

        # backbone: 8 groups of 16 kt slots; (pair, qc) interleaved. AVs are
        # APPENDED (never front-inserted): strict FIFO order guarantees the
        # V projection (Vones) is fully emitted before any AV reads it.
        order = [(0, 0), (1, 0), (0, 1), (1, 1), (0, 2), (1, 2), (0, 3), (1, 3)]
        ets = {}
        gas = {}
        gops = {}

        for gi, (p, qc) in enumerate(order):
            # group-entry guards
            ensure(need_qt(p, qc))               # QT (p, qc) projected
            if gi == 1:
                prefetch(xv_c, xv_ap, 0)
                prefetch(xv_c, xv_ap, 1)
                prefetch(xq_c, xq_ap, 1)
            if gi == 2:
                prefetch(xq_c, xq_ap, 2)
            if gi == 3:
                prefetch(xq_c, xq_ap, 3)
            # et buffer rotation (bufs=3): group gi reuses the buffer of
            # gi-3, whose AV must be fully drained first. Vones must be
            # complete before any AV pull (emission-order dataflow). Only
            # the AV's et-reading part (first 18 yields) gates the reuse;
            # its ZT transposes stay in the FIFO.
            if gi == 3:
                drain(gv)
            if gi >= 3:
                drain(gas[order[gi - 3]][0], 18)
            if gi == 4:
                # out-proj st-group 0 needs ZT qc0 from both pairs
                # ((0,0) drained at gi=3, (1,0) = order[1] drained above)
                gops[0] = gen_outproj(range(0, 4))
                fifo.append((gops[0], 0.55))
            if gi == 6:
                # gop1 needs (0,1) [drained gi=5] and (1,1) = order[3] [above]
                gops[1] = gen_outproj(range(4, 8))
                fifo.append((gops[1], 0.55))
            if gi == 7:
                # FIFO order alone guarantees gop2 is pulled only after
                # ga(1,2) is exhausted (it was appended earlier).
                gops[2] = gen_outproj(range(8, 12))
                fifo.append((gops[2], 0.55))

            et = ets.setdefault((p, qc), new_et())
            for kt in range(0, KT_TILES, 2):
                ensure(need_kt(p, (kt + 1) // 4))  # KT (p, sc) projected
                scores_kts(p, qc, et, [kt, kt + 1])
                # group 0's slots are DMA-paced (waiting on xk), so they
                # absorb extra projection emission without stretching ACT.
                pump(2.20 if gi == 0 else 1.60)
                if kt in (4, 8, 12) and gi >= 3 and v_prog[0] >= 16:
                    # spread the NEXT boundary's et-rotation drain (the AV
                    # of order[gi-2]) across this group in small chunks.
                    tgt = gas.get(order[gi - 2])
                    if tgt is not None:
                        drain(tgt[0], 5)

            # AV for this group appended as filler (after gv in FIFO order)
            ga = gen_av(p, qc, et)
            gas[(p, qc)] = (ga, 0.26)
            fifo.append((ga, 0.26))

        # tail: av(0,3)/av(1,3) + the out-proj groups that depend on them
        ga13 = gas[(1, 3)][0]
        fifo_drop(ga13)
        drain(gv)          # ppo safety: gop3 allocs must not interleave gv's
        drain(gas[(0, 3)][0])
        drain(ga13, 19)    # h-loops + qt0/qt1 transposes
        gop3 = gen_outproj(range(12, 16), act_evict=True)
        drain(gop3, 4)     # st12, st13
        drain(ga13)        # qt2/qt3 transposes
        while fifo:
            if drain(fifo[0][0], 4):
                fifo.pop(0)
        drain(gop3)        # st14, st15

    nc.compile()
    return nc


def prep_inputs(query, key, value, Wq, bq, Wk, bk, Wv, bv, Wo, bo):
    """Host-side sharding: per-core input dicts (bf16, transposed/augmented)."""
    bf = ml_dtypes.bfloat16
    q32 = np.asarray(query, np.float32)
    k32 = np.asarray(key, np.float32)
    v32 = np.asarray(value, np.float32)
    Wq = np.asarray(Wq, np.float32)
    Wk = np.asarray(Wk, np.float32)
    Wv = np.asarray(Wv, np.float32)
    Wo = np.asarray(Wo, np.float32)
    bq = np.asarray(bq, np.float32)
    bk = np.asarray(bk, np.float32)
    bv = np.asarray(bv, np.float32)

    scale = 1.0 / np.sqrt(np.float32(D))

    def xt_layout(x2d):
        # [S, E] -> X^T [E, S] -> [sc, half, p, c, j]: each [128, 4, 512]
        # half-tile is a fully contiguous DRAM block
        a = x2d.T.reshape(2, ET // 2, 128, QC, 512).transpose(3, 0, 2, 1, 4)
        return np.ascontiguousarray(a).astype(bf)

    def w_layout(w2d):
        # [E, D'] -> [p, eo, D'] contiguous
        a = w2d.reshape(ET, 128, w2d.shape[1]).transpose(1, 0, 2)
        return np.ascontiguousarray(a).astype(bf)

    def w_layout_pair(w2d):
        # [E, 256] -> [p, pair, eo, 128] contiguous (pair-major so pair-0
        # loads alone in the head)
        a = w2d.reshape(ET, 128, 2, 128).transpose(1, 2, 0, 3)
        return np.ascontiguousarray(a).astype(bf)

    xt = {}
    for b in range(B):
        xt[('q', b)] = xt_layout(q32[b])
        xt[('k', b)] = xt_layout(k32[b])
        xt[('v', b)] = xt_layout(v32[b])

    in_maps = []
    for c in range(N_CORES):
        b, g = c // HL, c % HL
        hs = slice(g * DL, (g + 1) * DL)
        wv_aug = np.zeros((E, HL * 65), np.float32)
        bv_aug = np.zeros((1, HL * 65), np.float32)
        for h in range(HL):
            wv_aug[:, h * 65:h * 65 + D] = Wv[:, g * DL + h * D: g * DL + (h + 1) * D]
            bv_aug[0, h * 65:h * 65 + D] = bv[g * DL + h * D: g * DL + (h + 1) * D]
            bv_aug[0, h * 65 + D] = 1.0
        in_maps.append({
            "xq_t": xt[('q', b)],
            "xk_t": xt[('k', b)],
            "xv_t": xt[('v', b)],
            "wq": w_layout_pair(Wq[:, hs] * scale),
            "wk": w_layout_pair(Wk[:, hs]),
            "wv": w_layout(wv_aug),
            "bq": np.ascontiguousarray(
                (bq[hs] * scale).reshape(2, 128).T).astype(np.float32),
            "bk": np.ascontiguousarray(
                bk[hs].reshape(2, 128).T).astype(np.float32),
            "bvf": np.ascontiguousarray(
                np.broadcast_to(bv_aug, (128, HL * 65))).astype(bf),
            "wo": np.ascontiguousarray(
                Wo[hs, :].reshape(2, 128, E).transpose(1, 0, 2)).astype(bf),
        })
    return in_maps


_NC_CACHE = [None]


def get_nc():
    if _NC_CACHE[0] is None:
        _install_ntff_hook()
        _NC_CACHE[0] = build_kernel()
    return _NC_CACHE[0]


def run(inputs, trace=False):
    nc = get_nc()
    in_maps = prep_inputs(**{k: v for k, v in inputs.items() if k != 'bo'},
                          bo=inputs['bo'])
    res = bass_utils.run_bass_kernel_spmd(
        nc, in_maps, core_ids=list(range(N_CORES)), trace=trace)
    bo = np.asarray(inputs['bo'], np.float32)
    out = np.empty((B, S, E), np.float32)
    for b in range(B):
        acc = np.zeros((S, E), np.float32)
        for g in range(HL):
            acc += np.asarray(res.results[b * HL + g]["out_p"], np.float32)
        out[b] = acc + bo[None, :]
    return out, res


def kernel(**inputs):
    out, _ = run(inputs, trace=False)
    return out
